# revision 60
# baseline (speedup 1.0000x reference)
"""MoE on 8 TRN2 cores — paired-expert F-split variant.

Experts are paired (largest token count with smallest); each pair of
experts (A, B) is assigned to two cores: core 2p takes the first half of
both experts' FFN dim, core 2p+1 the second half. Both cores process all
of A's and B's tokens over their F-half; the host sums the two partial
outputs. This halves the load-imbalance padding versus one-expert-per-core
and splits the per-core GEMM work nearly evenly.

Scheduling notes:
- Both stages run over EXACT token counts (no 128-padding anywhere):
  stage 2 keeps wt2 stationary and streams tokens as the moving dim, so
  its cost is proportional to real tokens. The gate is applied with an
  elementwise multiply against a host-broadcast [128, T] gate tile on
  the otherwise-idle vector engine; output leaves in [D, tokens] layout
  and the host transposes during combine.
- xet (tokens) is stored chunk-major in DRAM so each column-chunk load
  is one fully contiguous DMA; the first chunks are small so the
  cold-start pipeline stalls in small steps.
- gpsimd queue: xet chunks -> b1 -> wt2 quads -> gate tile. Sync queue:
  w1 stream + ys stores. Startup-critical loads never fight wt2 for
  bandwidth.
- PE warm-up: dependency-free dummy matmuls hold the tensor engine's
  DVFS clock at full speed through the DMA-bound first ~10us.
- Stage-1-lifetime pools are closed right after stage 1 so their
  release bookkeeping hides under stage-2 matmuls instead of
  serializing after the last matmul.
"""

import sys

import numpy as np

for _p in ("/opt/trn_rl_repo",):
    if _p not in sys.path:
        sys.path.append(_p)

import ml_dtypes
from contextlib import ExitStack

import concourse.bacc as bacc
import concourse.mybir as mybir
from concourse.tile import TileContext
from concourse.bass_utils import run_bass_kernel_spmd

D = 1024
F = 4096
F2 = F // 2
E = 8
TOP_K = 2
P = 128
DT = D // P    # 8 k-tiles for stage 1
FT = F // P    # 32 f tiles per core (16 per expert half)
FT2 = FT // 2
N_CORES = 8

BF16 = mybir.dt.bfloat16
F32 = mybir.dt.float32
NP_BF16 = ml_dtypes.bfloat16

_nc_cache = {}


def _round_up(v, m):
    return ((v + m - 1) // m) * m


def _chunks(total, size):
    out = []
    o = 0
    while o < total:
        out.append((o, min(size, total - o)))
        o += size
    return out


def _seg_chunks(s1, s2):
    """Stage-1 (col offset within segment, width) chunk lists == xet DMA
    chunks. Segment A leads with small chunks so the cold-start pipeline
    stalls in small steps."""
    a = [(0, 128), (128, 448)]
    a += [(o + 576, w) for o, w in _chunks(s1 - 576, 512)]
    b = _chunks(s2, 512)
    return [a, b]


def build_moe_nc(s1, s2, loop_n=1):
    """SPMD program: two expert-half FFNs over segmented tokens.

    Token columns [0, s1) belong to expert A, [s1, s1+s2) to expert B
    (exact counts). f-tiles 0..15 are A's F-half, 16..31 B's F-half.
    Output y is [D, s1+s2] (transposed; host untransposes).
    """
    assert loop_n == 1
    cpad = s1 + s2          # token columns (exact)
    # (token col offset, width, f-tile base)
    segs = [(0, s1, 0), (s1, s2, FT2)]
    seg_chunks = _seg_chunks(s1, s2)
    # stage-2 token chunks (decoupled from DMA chunks): near-even widths
    # with 128-element-aligned starts — a tiny chunk means a run of
    # matmuls with almost no rows, bound by sequencer pitch instead of
    # execution; odd starts risk slow SBUF access. The final chunk is
    # the smallest so the kernel-ending store chain is short.
    # Chunk starts MUST be multiples of 256 elements (512B in bf16):
    # 357- and 384-element starts both measured ~45us slower end-to-end.
    # (0,512),(512,256),(768,rest) keeps every start aligned while the
    # tail chunk stays wide enough to not be sequencer-pitch-bound.
    def _aligned_chunks(total):
        assert 768 < total <= 1280
        return [(0, 512), (512, 256), (768, total - 768)]

    s2_chunks = [_aligned_chunks(s1), _aligned_chunks(s2)]
    # free-dim offset of each chunk in the chunk-major xet layout
    xoff = [[], []]
    acc = 0
    for si in range(2):
        for (c0, cw) in seg_chunks[si]:
            xoff[si].append(acc)
            acc += DT * cw
    assert acc == DT * cpad

    nc = bacc.Bacc("TRN2", target_bir_lowering=False, debug=False,
                   num_devices=N_CORES)

    xet = nc.dram_tensor("xet", [P, DT * cpad], BF16, kind="ExternalInput")
    wt1 = nc.dram_tensor("wt1", [FT, P, DT * P], BF16, kind="ExternalInput")
    # quad-major: [quad, partition, 4 f-tiles * D]
    wt2 = nc.dram_tensor("wt2", [FT // 4, P, 4 * D], BF16, kind="ExternalInput")
    b1t = nc.dram_tensor("b1t", [P, FT], F32, kind="ExternalInput")
    gtb = nc.dram_tensor("gtb", [P, cpad], F32, kind="ExternalInput")
    yo = nc.dram_tensor("y", [D, cpad], BF16, kind="ExternalOutput")

    # h storage: A's 16 f-tiles get s1 columns each, B's get s2.
    def h_off(f):
        if f < FT2:
            return f * s1
        return FT2 * s1 + (f - FT2) * s2

    with TileContext(nc) as tc, ExitStack() as ctx:
        const = ctx.enter_context(tc.tile_pool(name="const", bufs=1))
        b1_sb = const.tile([P, FT], F32, tag="b1")
        gtb_sb = const.tile([P, cpad], F32, tag="gtb")

        # Long-lived pools first, then stage-1-lifetime pools in s1ctx
        # (closed right after stage 1, LIFO on top of the pool stack):
        # their releases land at the stage-1/stage-2 boundary, where the
        # PE sequencer hides them under stage-2 matmul execution instead
        # of serializing them after the last matmul at kernel end.
        w2pool = ctx.enter_context(tc.tile_pool(name="wt2", bufs=1))
        wt2_sb = w2pool.tile([P, FT * D], BF16, tag="wt2")
        hpool = ctx.enter_context(tc.tile_pool(name="h", bufs=1))
        h_all = hpool.tile([P, FT2 * (s1 + s2)], BF16, tag="h")
        ps2pool = ctx.enter_context(tc.tile_pool(name="ps2", bufs=4, space="PSUM"))
        ypool = ctx.enter_context(tc.tile_pool(name="ys", bufs=5))

        s1ctx = ExitStack()
        xpool = s1ctx.enter_context(tc.tile_pool(name="xet", bufs=1))
        xet_sb = xpool.tile([P, DT * cpad], BF16, tag="xet")
        w1pool = s1ctx.enter_context(tc.tile_pool(name="wt1", bufs=5))
        ps1pool = s1ctx.enter_context(
            tc.tile_pool(name="ps1", bufs=4, space="PSUM"))
        scratch = s1ctx.enter_context(tc.tile_pool(name="warm", bufs=1))

        # gpsimd queue order: A0, A1, b1, A2.., B0, B1, wt2 quads, gate.
        # The sync queue carries only the w1 stream (+ ys stores later),
        # so startup-critical loads never fight wt2 for bandwidth.
        def xchunk_dma(si, ci):
            o = xoff[si][ci]
            cw = seg_chunks[si][ci][1]
            nc.gpsimd.dma_start(
                out=xet_sb[:, o:o + DT * cw], in_=xet[:, o:o + DT * cw])

        xchunk_dma(0, 0)
        nc.gpsimd.dma_start(out=b1_sb[:], in_=b1t[:])
        for ci in range(2, len(seg_chunks[0])):
            xchunk_dma(0, ci)
        for ci in range(len(seg_chunks[1])):
            xchunk_dma(1, ci)
        # gate tile before the wt2 quads: stage 2's first vector op
        # needs it well before the B-half wt2 tiles are due.
        nc.gpsimd.dma_start(out=gtb_sb[:], in_=gtb[:])
        for q in range(FT // 4):
            nc.gpsimd.dma_start(
                out=wt2_sb[:, q * 4 * D:(q + 1) * 4 * D],
                in_=wt2[q, :, :])

        # PE warm-up: the tensor engine's clock ramps (0.65 -> 1.2 ->
        # 2.4 GHz) only while continuously busy. The first ~10us of
        # the kernel are DMA-bound, so run dependency-free dummy
        # matmuls on a never-DMA'd scratch tile to hold the clock
        # high until real operands arrive. Results are never read.
        wsrc = scratch.tile([P, 256], BF16, tag="wsrc")
        nc.vector.memset(wsrc[:], 0.0)
        # ~14 x ~250ns spans the gap between engine-preamble end (~7.7us)
        # and first operand arrival (~10.5us). Dummies rotate through the
        # ps1 pool so no dedicated PSUM bank is needed.
        for _ in range(14):
            wp = ps1pool.tile([P, 512], F32, tag="ps1")
            nc.tensor.matmul(wp[:, :256], wsrc[:, :128], wsrc[:],
                             start=True, stop=True)

        # Stage 1
        for si, (c_off, c_w, f_base) in enumerate(segs):
            for fi in range(FT2):
                f = f_base + fi
                w1f = w1pool.tile([P, DT * P], BF16, tag="w1f")
                nc.sync.dma_start(out=w1f[:], in_=wt1[f, :, :])
                if si == 0 and fi == 0:
                    # A1 rides the sync queue right behind w1f[0] so the
                    # two startup queues split the early xet burst.
                    o1 = xoff[0][1]
                    cw1 = seg_chunks[0][1][1]
                    nc.sync.dma_start(out=xet_sb[:, o1:o1 + DT * cw1],
                                      in_=xet[:, o1:o1 + DT * cw1])
                for ci, (c0, cw) in enumerate(seg_chunks[si]):
                    o = xoff[si][ci]
                    ps = ps1pool.tile([P, 512], F32, tag="ps1")
                    for dt in range(DT):
                        nc.tensor.matmul(
                            ps[:, :cw],
                            w1f[:, dt * P:(dt + 1) * P],
                            xet_sb[:, o + dt * cw:o + (dt + 1) * cw],
                            start=(dt == 0),
                            stop=(dt == DT - 1),
                        )
                    nc.scalar.activation(
                        h_all[:, h_off(f) + c0:h_off(f) + c0 + cw],
                        ps[:, :cw],
                        mybir.ActivationFunctionType.Gelu,
                        bias=b1_sb[:, f:f + 1],
                        scale=1.0,
                    )
        s1ctx.close()

        # Stage 2: wt2 stationary, tokens moving — out[d, t], exact
        # token widths. Gate applied via elementwise mul with the
        # broadcast gate tile; result stored transposed.
        for si, (c_off, c_w, f_base) in enumerate(segs):
            for (t0, tw) in s2_chunks[si]:
                for dti in range(D // P):
                    ps2 = ps2pool.tile([P, 512], F32, tag="ps2")
                    for fi in range(FT2):
                        f = f_base + fi
                        nc.tensor.matmul(
                            ps2[:, :tw],
                            wt2_sb[:, f * D + dti * P:f * D + (dti + 1) * P],
                            h_all[:, h_off(f) + t0:h_off(f) + t0 + tw],
                            start=(fi == 0),
                            stop=(fi == FT2 - 1),
                        )
                    ysd = ypool.tile([P, 512], BF16, tag="ys")
                    nc.vector.tensor_mul(
                        ysd[:, :tw], ps2[:, :tw],
                        gtb_sb[:, c_off + t0:c_off + t0 + tw])
                    nc.sync.dma_start(
                        out=yo[dti * P:(dti + 1) * P,
                               c_off + t0:c_off + t0 + tw],
                        in_=ysd[:, :tw])

    nc.compile()
    return nc


def _get_nc(s1, s2, loop_n=1):
    key = (s1, s2, loop_n)
    if key not in _nc_cache:
        _nc_cache[key] = build_moe_nc(s1, s2, loop_n)
    return _nc_cache[key]


def _route(xf, Wr):
    logits = xf.astype(np.float64) @ Wr.astype(np.float64).T
    order = np.argsort(-logits, axis=1, kind="stable")
    top_i = order[:, :TOP_K]
    top_l = np.take_along_axis(logits, top_i, axis=1)
    m = top_l.max(axis=1, keepdims=True)
    ex = np.exp(top_l - m)
    gate = (ex / ex.sum(axis=1, keepdims=True)).astype(np.float32)
    return top_i, gate


def _tile_w1(block_bf):
    """[F2, D] bf16 -> [FT2, P, DT*P] so each f-tile DMA is contiguous."""
    return np.ascontiguousarray(
        block_bf.reshape(FT2, P, DT, P).transpose(0, 3, 2, 1)
    ).reshape(FT2, P, DT * P)


def make_in_maps(x, Wr, W1, b1, W2, b2):
    B, S, _ = x.shape
    T = B * S
    xf = np.asarray(x, dtype=np.float32).reshape(T, D)
    top_i, gate = _route(xf, np.asarray(Wr, dtype=np.float32))

    idx_list, gate_list = [], []
    for e in range(E):
        t_idx, k_idx = np.nonzero(top_i == e)
        idx_list.append(t_idx.astype(np.int64))
        gate_list.append(gate[t_idx, k_idx])

    counts = np.array([len(i) for i in idx_list])
    order = np.argsort(-counts, kind="stable")
    pairs = [(int(order[i]), int(order[7 - i])) for i in range(4)]
    s1 = max(max(int(counts[a]), 1) for a, _ in pairs)
    s2 = max(max(int(counts[b]), 1) for _, b in pairs)
    cpad = s1 + s2
    seg_chunks = _seg_chunks(s1, s2)

    xfT = np.ascontiguousarray(xf.T).astype(NP_BF16)
    W1bf = np.asarray(W1, dtype=np.float32).astype(NP_BF16)   # [E, F, D]
    W2bf = np.asarray(W2, dtype=np.float32).astype(NP_BF16)   # [E, D, F]
    b1f = np.asarray(b1, dtype=np.float32)

    in_maps = []
    for p, (a, b) in enumerate(pairs):
        xe = np.zeros((D, cpad), dtype=NP_BF16)
        xe[:, :counts[a]] = xfT[:, idx_list[a]]
        xe[:, s1:s1 + counts[b]] = xfT[:, idx_list[b]]
        # [D, cpad] -> [P, DT, cpad] -> chunk-major [P, DT*cpad]
        xe3 = xe.reshape(DT, P, cpad).transpose(1, 0, 2)
        blocks = []
        for si, c_base in ((0, 0), (1, s1)):
            for (c0, cw) in seg_chunks[si]:
                a0 = c_base + c0
                blocks.append(np.ascontiguousarray(
                    xe3[:, :, a0:a0 + cw]).reshape(P, DT * cw))
        xet = np.ascontiguousarray(np.concatenate(blocks, axis=1))
        gv = np.zeros(cpad, dtype=np.float32)
        gv[:counts[a]] = gate_list[a]
        gv[s1:s1 + counts[b]] = gate_list[b]
        gtb = np.ascontiguousarray(np.broadcast_to(gv[None, :], (P, cpad)))
        for h in range(2):
            fsl = slice(h * F2, (h + 1) * F2)
            wt1 = np.concatenate(
                [_tile_w1(W1bf[a][fsl, :]), _tile_w1(W1bf[b][fsl, :])], axis=0)
            wt2f = np.concatenate(
                [W2bf[a][:, fsl].T, W2bf[b][:, fsl].T], axis=0)  # [F, D]
            # quad-major: [FT//4, P, 4*D]
            wt2 = np.ascontiguousarray(
                wt2f.reshape(FT // 4, 4, P, D).transpose(0, 2, 1, 3)
            ).reshape(FT // 4, P, 4 * D)
            b1c = np.concatenate(
                [b1f[a][fsl].reshape(FT2, P).T, b1f[b][fsl].reshape(FT2, P).T],
                axis=1)
            in_maps.append({
                "xet": xet,
                "wt1": wt1,
                "wt2": wt2,
                "b1t": np.ascontiguousarray(b1c),
                "gtb": gtb,
            })
    meta = dict(pairs=pairs, s1=s1, s2=s2,
                idx_list=idx_list, top_i=top_i, gate=gate, counts=counts)
    return in_maps, meta


def combine(results, meta, x_shape, b2):
    B, S, _ = x_shape
    T = B * S
    s1 = meta["s1"]
    counts = meta["counts"]
    idx_list = meta["idx_list"]
    out = np.zeros((T, D), dtype=np.float32)
    for p, (a, b) in enumerate(meta["pairs"]):
        ya = (results[2 * p]["y"].astype(np.float32)
              + results[2 * p + 1]["y"].astype(np.float32))   # [D, cpad]
        if counts[a]:
            out[idx_list[a]] += ya[:, :counts[a]].T
        if counts[b]:
            out[idx_list[b]] += ya[:, s1:s1 + counts[b]].T
    b2 = np.asarray(b2, dtype=np.float32)
    if np.any(b2):
        comb = np.zeros((T, E), dtype=np.float32)
        comb[np.arange(T)[:, None], meta["top_i"]] = meta["gate"]
        out += comb @ b2
    return out.reshape(B, S, D)


def kernel(x, Wr, W1, b1, W2, b2):
    in_maps, meta = make_in_maps(x, Wr, W1, b1, W2, b2)
    nc = _get_nc(meta["s1"], meta["s2"])
    res = run_bass_kernel_spmd(nc, in_maps, list(range(N_CORES)))
    return combine(res.results, meta, x.shape, b2)


# revision 61
# speedup vs baseline: 1.0171x; 1.0171x over previous
"""MoE on 8 TRN2 cores — paired-expert F-split variant.

Experts are paired (largest token count with smallest); each pair of
experts (A, B) is assigned to two cores: core 2p takes the first half of
both experts' FFN dim, core 2p+1 the second half. Both cores process all
of A's and B's tokens over their F-half; the host sums the two partial
outputs. This halves the load-imbalance padding versus one-expert-per-core
and splits the per-core GEMM work nearly evenly.

Scheduling notes:
- Both stages run over EXACT token counts (no 128-padding anywhere):
  stage 2 keeps wt2 stationary and streams tokens as the moving dim, so
  its cost is proportional to real tokens. The gate is applied with an
  elementwise multiply against a host-broadcast [128, T] gate tile on
  the otherwise-idle vector engine; output leaves in [D, tokens] layout
  and the host transposes during combine.
- xet (tokens) is stored chunk-major in DRAM so each column-chunk load
  is one fully contiguous DMA; the first chunks are small so the
  cold-start pipeline stalls in small steps.
- gpsimd queue: xet chunks -> b1 -> wt2 quads -> gate tile. Sync queue:
  w1 stream + ys stores. Startup-critical loads never fight wt2 for
  bandwidth.
- PE warm-up: dependency-free dummy matmuls hold the tensor engine's
  DVFS clock at full speed through the DMA-bound first ~10us.
- Stage-1-lifetime pools are closed right after stage 1 so their
  release bookkeeping hides under stage-2 matmuls instead of
  serializing after the last matmul.
"""

import sys

import numpy as np

for _p in ("/opt/trn_rl_repo",):
    if _p not in sys.path:
        sys.path.append(_p)

import ml_dtypes
from contextlib import ExitStack

import concourse.bacc as bacc
import concourse.mybir as mybir
from concourse.tile import TileContext
from concourse.bass_utils import run_bass_kernel_spmd

D = 1024
F = 4096
F2 = F // 2
E = 8
TOP_K = 2
P = 128
DT = D // P    # 8 k-tiles for stage 1
FT = F // P    # 32 f tiles per core (16 per expert half)
FT2 = FT // 2
N_CORES = 8

BF16 = mybir.dt.bfloat16
F32 = mybir.dt.float32
NP_BF16 = ml_dtypes.bfloat16

_nc_cache = {}


def _round_up(v, m):
    return ((v + m - 1) // m) * m


def _chunks(total, size):
    out = []
    o = 0
    while o < total:
        out.append((o, min(size, total - o)))
        o += size
    return out


def _seg_chunks(s1, s2):
    """Stage-1 (col offset within segment, width) chunk lists == xet DMA
    chunks. Segment A leads with small chunks so the cold-start pipeline
    stalls in small steps."""
    a = [(0, 128), (128, 448), (576, 256)]
    a += [(o + 832, w) for o, w in _chunks(s1 - 832, 512)]
    b = _chunks(s2, 512)
    return [a, b]


def build_moe_nc(s1, s2, loop_n=1):
    """SPMD program: two expert-half FFNs over segmented tokens.

    Token columns [0, s1) belong to expert A, [s1, s1+s2) to expert B
    (exact counts). f-tiles 0..15 are A's F-half, 16..31 B's F-half.
    Output y is [D, s1+s2] (transposed; host untransposes).
    """
    assert loop_n == 1
    cpad = s1 + s2          # token columns (exact)
    # (token col offset, width, f-tile base)
    segs = [(0, s1, 0), (s1, s2, FT2)]
    seg_chunks = _seg_chunks(s1, s2)
    # stage-2 token chunks (decoupled from DMA chunks): near-even widths
    # with 128-element-aligned starts — a tiny chunk means a run of
    # matmuls with almost no rows, bound by sequencer pitch instead of
    # execution; odd starts risk slow SBUF access. The final chunk is
    # the smallest so the kernel-ending store chain is short.
    # Chunk starts MUST be multiples of 256 elements (512B in bf16):
    # 357- and 384-element starts both measured ~45us slower end-to-end.
    # (0,512),(512,256),(768,rest) keeps every start aligned while the
    # tail chunk stays wide enough to not be sequencer-pitch-bound.
    def _aligned_chunks(total):
        assert 768 < total <= 1280
        return [(0, 512), (512, 256), (768, total - 768)]

    s2_chunks = [_aligned_chunks(s1), _aligned_chunks(s2)]
    # free-dim offset of each chunk in the chunk-major xet layout
    xoff = [[], []]
    acc = 0
    for si in range(2):
        for (c0, cw) in seg_chunks[si]:
            xoff[si].append(acc)
            acc += DT * cw
    assert acc == DT * cpad

    nc = bacc.Bacc("TRN2", target_bir_lowering=False, debug=False,
                   num_devices=N_CORES)

    xet = nc.dram_tensor("xet", [P, DT * cpad], BF16, kind="ExternalInput")
    wt1 = nc.dram_tensor("wt1", [FT, P, DT * P], BF16, kind="ExternalInput")
    # quad-major: [quad, partition, 4 f-tiles * D]
    wt2 = nc.dram_tensor("wt2", [FT // 4, P, 4 * D], BF16, kind="ExternalInput")
    b1t = nc.dram_tensor("b1t", [P, FT], F32, kind="ExternalInput")
    gtb = nc.dram_tensor("gtb", [P, cpad], F32, kind="ExternalInput")
    yo = nc.dram_tensor("y", [D, cpad], BF16, kind="ExternalOutput")

    # h storage: A's 16 f-tiles get s1 columns each, B's get s2.
    def h_off(f):
        if f < FT2:
            return f * s1
        return FT2 * s1 + (f - FT2) * s2

    with TileContext(nc) as tc, ExitStack() as ctx:
        const = ctx.enter_context(tc.tile_pool(name="const", bufs=1))
        b1_sb = const.tile([P, FT], F32, tag="b1")
        gtb_sb = const.tile([P, cpad], F32, tag="gtb")

        # Long-lived pools first, then stage-1-lifetime pools in s1ctx
        # (closed right after stage 1, LIFO on top of the pool stack):
        # their releases land at the stage-1/stage-2 boundary, where the
        # PE sequencer hides them under stage-2 matmul execution instead
        # of serializing them after the last matmul at kernel end.
        w2pool = ctx.enter_context(tc.tile_pool(name="wt2", bufs=1))
        wt2_sb = w2pool.tile([P, FT * D], BF16, tag="wt2")
        hpool = ctx.enter_context(tc.tile_pool(name="h", bufs=1))
        h_all = hpool.tile([P, FT2 * (s1 + s2)], BF16, tag="h")
        ps2pool = ctx.enter_context(tc.tile_pool(name="ps2", bufs=4, space="PSUM"))
        ypool = ctx.enter_context(tc.tile_pool(name="ys", bufs=5))

        s1ctx = ExitStack()
        xpool = s1ctx.enter_context(tc.tile_pool(name="xet", bufs=1))
        xet_sb = xpool.tile([P, DT * cpad], BF16, tag="xet")
        w1pool = s1ctx.enter_context(tc.tile_pool(name="wt1", bufs=5))
        ps1pool = s1ctx.enter_context(
            tc.tile_pool(name="ps1", bufs=4, space="PSUM"))
        scratch = s1ctx.enter_context(tc.tile_pool(name="warm", bufs=1))

        # gpsimd queue order: A0, A1, b1, A2.., B0, B1, wt2 quads, gate.
        # The sync queue carries only the w1 stream (+ ys stores later),
        # so startup-critical loads never fight wt2 for bandwidth.
        def xchunk_dma(si, ci):
            o = xoff[si][ci]
            cw = seg_chunks[si][ci][1]
            nc.gpsimd.dma_start(
                out=xet_sb[:, o:o + DT * cw], in_=xet[:, o:o + DT * cw])

        xchunk_dma(0, 0)
        nc.gpsimd.dma_start(out=b1_sb[:], in_=b1t[:])
        for ci in range(2, len(seg_chunks[0])):
            xchunk_dma(0, ci)
        for ci in range(len(seg_chunks[1])):
            xchunk_dma(1, ci)
        # gate tile before the wt2 quads: stage 2's first vector op
        # needs it well before the B-half wt2 tiles are due.
        nc.gpsimd.dma_start(out=gtb_sb[:], in_=gtb[:])
        for q in range(FT // 4):
            nc.gpsimd.dma_start(
                out=wt2_sb[:, q * 4 * D:(q + 1) * 4 * D],
                in_=wt2[q, :, :])

        # PE warm-up: the tensor engine's clock ramps (0.65 -> 1.2 ->
        # 2.4 GHz) only while continuously busy. The first ~10us of
        # the kernel are DMA-bound, so run dependency-free dummy
        # matmuls on a never-DMA'd scratch tile to hold the clock
        # high until real operands arrive. Results are never read.
        wsrc = scratch.tile([P, 256], BF16, tag="wsrc")
        nc.vector.memset(wsrc[:], 0.0)
        # ~14 x ~250ns spans the gap between engine-preamble end (~7.7us)
        # and first operand arrival (~10.5us). Dummies rotate through the
        # ps1 pool so no dedicated PSUM bank is needed.
        for _ in range(14):
            wp = ps1pool.tile([P, 512], F32, tag="ps1")
            nc.tensor.matmul(wp[:, :256], wsrc[:, :128], wsrc[:],
                             start=True, stop=True)

        # Stage 1
        for si, (c_off, c_w, f_base) in enumerate(segs):
            for fi in range(FT2):
                f = f_base + fi
                w1f = w1pool.tile([P, DT * P], BF16, tag="w1f")
                nc.sync.dma_start(out=w1f[:], in_=wt1[f, :, :])
                if si == 0 and fi == 0:
                    # A1 rides the sync queue right behind w1f[0] so the
                    # two startup queues split the early xet burst.
                    o1 = xoff[0][1]
                    cw1 = seg_chunks[0][1][1]
                    nc.sync.dma_start(out=xet_sb[:, o1:o1 + DT * cw1],
                                      in_=xet[:, o1:o1 + DT * cw1])
                for ci, (c0, cw) in enumerate(seg_chunks[si]):
                    o = xoff[si][ci]
                    ps = ps1pool.tile([P, 512], F32, tag="ps1")
                    for dt in range(DT):
                        nc.tensor.matmul(
                            ps[:, :cw],
                            w1f[:, dt * P:(dt + 1) * P],
                            xet_sb[:, o + dt * cw:o + (dt + 1) * cw],
                            start=(dt == 0),
                            stop=(dt == DT - 1),
                        )
                    nc.scalar.activation(
                        h_all[:, h_off(f) + c0:h_off(f) + c0 + cw],
                        ps[:, :cw],
                        mybir.ActivationFunctionType.Gelu,
                        bias=b1_sb[:, f:f + 1],
                        scale=1.0,
                    )
        s1ctx.close()

        # Stage 2: wt2 stationary, tokens moving — out[d, t], exact
        # token widths. Gate applied via elementwise mul with the
        # broadcast gate tile; result stored transposed.
        for si, (c_off, c_w, f_base) in enumerate(segs):
            for (t0, tw) in s2_chunks[si]:
                for dti in range(D // P):
                    ps2 = ps2pool.tile([P, 512], F32, tag="ps2")
                    for fi in range(FT2):
                        f = f_base + fi
                        nc.tensor.matmul(
                            ps2[:, :tw],
                            wt2_sb[:, f * D + dti * P:f * D + (dti + 1) * P],
                            h_all[:, h_off(f) + t0:h_off(f) + t0 + tw],
                            start=(fi == 0),
                            stop=(fi == FT2 - 1),
                        )
                    ysd = ypool.tile([P, 512], BF16, tag="ys")
                    nc.vector.tensor_mul(
                        ysd[:, :tw], ps2[:, :tw],
                        gtb_sb[:, c_off + t0:c_off + t0 + tw])
                    nc.sync.dma_start(
                        out=yo[dti * P:(dti + 1) * P,
                               c_off + t0:c_off + t0 + tw],
                        in_=ysd[:, :tw])

    nc.compile()
    return nc


def _get_nc(s1, s2, loop_n=1):
    key = (s1, s2, loop_n)
    if key not in _nc_cache:
        _nc_cache[key] = build_moe_nc(s1, s2, loop_n)
    return _nc_cache[key]


def _route(xf, Wr):
    logits = xf.astype(np.float64) @ Wr.astype(np.float64).T
    order = np.argsort(-logits, axis=1, kind="stable")
    top_i = order[:, :TOP_K]
    top_l = np.take_along_axis(logits, top_i, axis=1)
    m = top_l.max(axis=1, keepdims=True)
    ex = np.exp(top_l - m)
    gate = (ex / ex.sum(axis=1, keepdims=True)).astype(np.float32)
    return top_i, gate


def _tile_w1(block_bf):
    """[F2, D] bf16 -> [FT2, P, DT*P] so each f-tile DMA is contiguous."""
    return np.ascontiguousarray(
        block_bf.reshape(FT2, P, DT, P).transpose(0, 3, 2, 1)
    ).reshape(FT2, P, DT * P)


def make_in_maps(x, Wr, W1, b1, W2, b2):
    B, S, _ = x.shape
    T = B * S
    xf = np.asarray(x, dtype=np.float32).reshape(T, D)
    top_i, gate = _route(xf, np.asarray(Wr, dtype=np.float32))

    idx_list, gate_list = [], []
    for e in range(E):
        t_idx, k_idx = np.nonzero(top_i == e)
        idx_list.append(t_idx.astype(np.int64))
        gate_list.append(gate[t_idx, k_idx])

    counts = np.array([len(i) for i in idx_list])
    order = np.argsort(-counts, kind="stable")
    pairs = [(int(order[i]), int(order[7 - i])) for i in range(4)]
    s1 = max(max(int(counts[a]), 1) for a, _ in pairs)
    s2 = max(max(int(counts[b]), 1) for _, b in pairs)
    cpad = s1 + s2
    seg_chunks = _seg_chunks(s1, s2)

    xfT = np.ascontiguousarray(xf.T).astype(NP_BF16)
    W1bf = np.asarray(W1, dtype=np.float32).astype(NP_BF16)   # [E, F, D]
    W2bf = np.asarray(W2, dtype=np.float32).astype(NP_BF16)   # [E, D, F]
    b1f = np.asarray(b1, dtype=np.float32)

    in_maps = []
    for p, (a, b) in enumerate(pairs):
        xe = np.zeros((D, cpad), dtype=NP_BF16)
        xe[:, :counts[a]] = xfT[:, idx_list[a]]
        xe[:, s1:s1 + counts[b]] = xfT[:, idx_list[b]]
        # [D, cpad] -> [P, DT, cpad] -> chunk-major [P, DT*cpad]
        xe3 = xe.reshape(DT, P, cpad).transpose(1, 0, 2)
        blocks = []
        for si, c_base in ((0, 0), (1, s1)):
            for (c0, cw) in seg_chunks[si]:
                a0 = c_base + c0
                blocks.append(np.ascontiguousarray(
                    xe3[:, :, a0:a0 + cw]).reshape(P, DT * cw))
        xet = np.ascontiguousarray(np.concatenate(blocks, axis=1))
        gv = np.zeros(cpad, dtype=np.float32)
        gv[:counts[a]] = gate_list[a]
        gv[s1:s1 + counts[b]] = gate_list[b]
        gtb = np.ascontiguousarray(np.broadcast_to(gv[None, :], (P, cpad)))
        for h in range(2):
            fsl = slice(h * F2, (h + 1) * F2)
            wt1 = np.concatenate(
                [_tile_w1(W1bf[a][fsl, :]), _tile_w1(W1bf[b][fsl, :])], axis=0)
            wt2f = np.concatenate(
                [W2bf[a][:, fsl].T, W2bf[b][:, fsl].T], axis=0)  # [F, D]
            # quad-major: [FT//4, P, 4*D]
            wt2 = np.ascontiguousarray(
                wt2f.reshape(FT // 4, 4, P, D).transpose(0, 2, 1, 3)
            ).reshape(FT // 4, P, 4 * D)
            b1c = np.concatenate(
                [b1f[a][fsl].reshape(FT2, P).T, b1f[b][fsl].reshape(FT2, P).T],
                axis=1)
            in_maps.append({
                "xet": xet,
                "wt1": wt1,
                "wt2": wt2,
                "b1t": np.ascontiguousarray(b1c),
                "gtb": gtb,
            })
    meta = dict(pairs=pairs, s1=s1, s2=s2,
                idx_list=idx_list, top_i=top_i, gate=gate, counts=counts)
    return in_maps, meta


def combine(results, meta, x_shape, b2):
    B, S, _ = x_shape
    T = B * S
    s1 = meta["s1"]
    counts = meta["counts"]
    idx_list = meta["idx_list"]
    out = np.zeros((T, D), dtype=np.float32)
    for p, (a, b) in enumerate(meta["pairs"]):
        ya = (results[2 * p]["y"].astype(np.float32)
              + results[2 * p + 1]["y"].astype(np.float32))   # [D, cpad]
        if counts[a]:
            out[idx_list[a]] += ya[:, :counts[a]].T
        if counts[b]:
            out[idx_list[b]] += ya[:, s1:s1 + counts[b]].T
    b2 = np.asarray(b2, dtype=np.float32)
    if np.any(b2):
        comb = np.zeros((T, E), dtype=np.float32)
        comb[np.arange(T)[:, None], meta["top_i"]] = meta["gate"]
        out += comb @ b2
    return out.reshape(B, S, D)


def kernel(x, Wr, W1, b1, W2, b2):
    in_maps, meta = make_in_maps(x, Wr, W1, b1, W2, b2)
    nc = _get_nc(meta["s1"], meta["s2"])
    res = run_bass_kernel_spmd(nc, in_maps, list(range(N_CORES)))
    return combine(res.results, meta, x.shape, b2)
